# revision 20
# baseline (speedup 1.0000x reference)
"""ConvCrossAttention Trainium2 kernel — self-contained.

Problem (B=4, C_in=C_out=256, H=W=64, N=4096):
  q = conv1x1(x1, Wq, bq); k = conv1x1(x2, Wk, bk); v = conv1x1(x2, Wv, bv)
  out = softmax(q^T k / sqrt(C)) @ v^T, back in conv layout [B, C, H, W].

Sharding: data-parallel over (batch, query-half) -> 8 NeuronCores.
Core c handles batch c//2, query rows (c%2)*2048 : (c%2+1)*2048, with the
full 4096-key context for that batch. No collectives.

Weight fusion (host side): softmax over keys is invariant to per-query
additive constants, so
  S_nm =(softmax) (A^T x1_n + c) . x2_m   with A = Wq^T Wk, c = Wk^T bq.
The K projection disappears entirely (raw x2 is the key matrix) and the
q projection uses the fused A instead of Wq. All inputs are pre-split on
the host to the [128 partitions, 2 halves, w] SBUF layout so every DMA
moves large contiguous per-partition runs (2KB descriptors starved
behind the 4-8KB x2 quarters otherwise).

Per-core program (everything SBUF-resident):
  Warm-up: fp32 dummy matmuls during the input-DMA head keep the PE busy
  so the HAM clock gate reaches 8/8 (2.4 GHz) before real work.
  DMA: three descriptor rings (Sync / Activation / Pool), each leading
  with its critical tensor: [aT, x1, x2 q4], [wv, x2 q2, bv], [cq, x2 q1,
  x2 q3].
  Prologue: all 16 V-pair projections (fp8 out) + the 4 q projections
  stream behind the DMA.
  Main loop: 4 query chunks, flash-style, in 2-key-tile pairs: S^T =
  x2^T q (PE, f32r, two banks of one wide PSUM slot), P = exp(S/16)
  (ACT, ONE 1024-wide activation per pair, fp8e4 out; |scores|/16 < ~5
  so no max-subtraction, p_max << 448), PV via fp8 DoubleRow matmuls
  (one matmul per 256 keys). P-sums ride the idle DVE (fp8-in adds;
  GpSimd is ~5x slower on fp8 and PE DoubleRow reductions cost a full
  matmul slot - both measured). The previous chunk's trailing PV pairs
  and its tail (denominator matmul, bv-fold matmuls closing the
  accumulation, reciprocal, Pool-engine partition broadcast, normalize,
  out-DMA) are woven into pairs 1..3 of the next chunk's S stream so the
  in-order PE queue never stalls.

S matmuls stay float32r (1 cyc/row); dropping scores to fp8 would cost
~1.7e-2 relative error (measured off-line) against the 2e-2 budget, while
fp8 P/V costs only ~1e-2. Softmax denominators use reciprocal_approx_fast
(~18-bit); inputs are sums of positive exps so its undefined edge cases
(0/denorm/inf) cannot occur.
"""

import sys

if "/opt/trn_rl_repo" not in sys.path:
    sys.path.insert(0, "/opt/trn_rl_repo")

from contextlib import ExitStack

import numpy as np

import concourse.bass as bass  # noqa: F401
import concourse.mybir as mybir
import concourse.tile as tile
from concourse import bacc
from concourse.bass_utils import run_bass_kernel_spmd

F32 = mybir.dt.float32
F32R = mybir.dt.float32r
F16 = mybir.dt.float16
F8 = mybir.dt.float8e4
DR = mybir.MatmulPerfMode.DoubleRow

B, C, H, W = 4, 256, 64, 64
N = H * W  # 4096
NQ = 2048  # queries per core (half a batch)
NK = 4096  # full key context
CHUNK = 512
NQ_CHUNKS = NQ // CHUNK
NK_TILES = NK // 128  # 32
NPAIRS = NK_TILES // 2  # 16 fp8 DoubleRow PV pairs
XQ = 1024  # x2 DMA quarter width
SCALE = 1.0 / 16.0  # C ** -0.5
WARMUP_MMS = 3  # fp32 dummy matmuls (~1.7us each) bridging the DMA head
PVTRAIL = 4  # PV pairs trail S pairs by this much


def build_nc():
    MM = F32R
    nc = bacc.Bacc(None, debug=False)

    # all pre-split host-side: [128, 2, w]
    x1 = nc.dram_tensor("x1c", [128, 2, NQ], MM, kind="ExternalInput")
    x2g = [
        nc.dram_tensor(f"x2{g}", [128, 2, XQ], MM, kind="ExternalInput")
        for g in range(NK // XQ)
    ]
    at = nc.dram_tensor("aT", [128, 2, C], MM, kind="ExternalInput")  # A = Wq^T Wk
    wv = nc.dram_tensor("wvT", [128, 2, C], MM, kind="ExternalInput")
    cq = nc.dram_tensor("cq", [128, 2, 1], F32, kind="ExternalInput")  # Wk^T bq
    bv = nc.dram_tensor("bv", [C, 1], F32, kind="ExternalInput")
    out = nc.dram_tensor("out", [C, NQ], F32, kind="ExternalOutput")

    def split_h(ap):  # DRAM [256, w] -> [128, 2, w] (partition-first)
        return ap.rearrange("(h p) w -> p h w", p=128)

    with tile.TileContext(nc) as tc, ExitStack() as ctx:
        big = ctx.enter_context(tc.tile_pool(name="big", bufs=1))
        small = ctx.enter_context(tc.tile_pool(name="small", bufs=1))
        ppool = ctx.enter_context(tc.tile_pool(name="p", bufs=6))
        opool = ctx.enter_context(tc.tile_pool(name="o", bufs=2))
        dpool = ctx.enter_context(tc.tile_pool(name="d", bufs=2))
        # PSUM: 2 wide S slots (2 banks each) + 3 acc + 1 den = 8 banks
        spsum = ctx.enter_context(tc.tile_pool(name="spsum", bufs=2, space="PSUM"))
        apsum = ctx.enter_context(tc.tile_pool(name="apsum", bufs=3, space="PSUM"))
        dpsum = ctx.enter_context(tc.tile_pool(name="dpsum", bufs=1, space="PSUM"))

        # --- SBUF residents ---
        a_sb = small.tile([128, 2, C], MM, tag="a")
        wv_sb = small.tile([128, 2, C], MM, tag="wv")
        cq_sb = small.tile([128, 2, 1], F32, tag="cq")
        x1_sb = big.tile([128, 2, NQ], MM, tag="x1")
        x2_sb = big.tile([128, 2, NK], MM, tag="x2")
        q_sb = big.tile([128, 2, NQ], MM, tag="q")
        v_sb = big.tile([128, NPAIRS, 2, C], F8, tag="v")
        wu = small.tile([128, 512], F32, tag="wu")
        ones_col_f32 = small.tile([128, 1], F32, tag="ones_col_f32")
        ones_row_f32 = small.tile([1, 128], F32, tag="ones_row_f32")
        ones_col = small.tile([128, 1], MM, tag="ones_col")
        ones_row = small.tile([1, 128], MM, tag="ones_row")
        bv_row = small.tile([1, 2, 128], MM, tag="bv_row")
        bv_bcast = small.tile([128, 2 * 128], MM, tag="bv_bcast")

        # --- DMA triggers. Three rings (Sync / Activation HWDGE, Pool
        # SWDGE) drain in FIFO order each, so every ring leads with its
        # critical tensor. ---
        nc.gpsimd.memset(wu[:], 0.0)
        nc.gpsimd.dma_start(out=cq_sb[:], in_=cq[:])
        nc.gpsimd.dma_start(out=x2_sb[:, :, 0:XQ], in_=x2g[0][:])
        nc.gpsimd.memset(ones_col_f32[:], 1.0)
        nc.gpsimd.memset(ones_row_f32[:], 1.0)
        nc.vector.tensor_copy(ones_col[:], ones_col_f32[:])
        nc.vector.tensor_copy(ones_row[:], ones_row_f32[:])
        nc.gpsimd.dma_start(out=x2_sb[:, :, 2 * XQ : 3 * XQ], in_=x2g[2][:])

        nc.sync.dma_start(out=a_sb[:], in_=at[:])
        nc.sync.dma_start(out=x1_sb[:], in_=x1[:])
        nc.sync.dma_start(out=x2_sb[:, :, 3 * XQ : 4 * XQ], in_=x2g[3][:])

        nc.scalar.dma_start(out=wv_sb[:], in_=wv[:])
        nc.scalar.dma_start(out=x2_sb[:, :, XQ : 2 * XQ], in_=x2g[1][:])
        nc.scalar.dma_start(
            out=bv_row[:], in_=bv[:, :].rearrange("(h p) o -> o h p", p=128).bitcast(F32R)
        )

        # --- HAM warm-up: fp32 dummy matmuls (4 cyc/row, ~1.7us each)
        # with no input dependency bridge the DMA head so the PE reaches
        # the 8/8 clock before, and stays busy until, real work starts ---
        for _ in range(WARMUP_MMS):
            wup = dpsum.tile([128, 512], F32, tag="db", name="wup")
            nc.tensor.matmul(wup[:], wu[:, 0:128], wu[:], start=True, stop=True)

        # --- projection helpers (prologue; share the wide S slots one
        # bank at a time) ---
        def qproj(c0):
            cs = slice(c0 * CHUNK, (c0 + 1) * CHUNK)
            for ct in range(2):
                qp = spsum.tile([128, 2, CHUNK], F32, tag="s", name="qp")
                cts = slice(ct * 128, (ct + 1) * 128)
                nc.tensor.matmul(qp[:, 0, :], a_sb[:, 0, cts], x1_sb[:, 0, cs], start=True, stop=False)
                nc.tensor.matmul(qp[:, 0, :], a_sb[:, 1, cts], x1_sb[:, 1, cs], start=False, stop=True)
                nc.vector.tensor_scalar_add(q_sb[:, ct, cs], qp[:, 0, :], cq_sb[:, ct, :])

        def vproj_pair(u):
            # two 128-key tiles share one PSUM bank -> one wide fp8 copy out
            vp = spsum.tile([128, 2, C], F32, tag="s", name="vp")
            for s2 in range(2):
                t = 2 * u + s2
                ts_ = slice(t * 128, (t + 1) * 128)
                nc.tensor.matmul(
                    vp[:, s2, :], x2_sb[:, 0, ts_], wv_sb[:, 0, :],
                    start=(s2 == 0), stop=False,
                )
                nc.tensor.matmul(
                    vp[:, s2, :], x2_sb[:, 1, ts_], wv_sb[:, 1, :],
                    start=False, stop=(s2 == 1),
                )
            if u % 2 == 0:
                nc.scalar.copy(v_sb[:, u, :, :], vp[:])
            else:
                nc.vector.tensor_copy(v_sb[:, u, :, :], vp[:])

        def bv_setup():
            # bv broadcast to all partitions: bias-fold matmul stationary
            # (acc_ct += bv_ct (x) den, so no per-half DVE bias add needed)
            bvb_ps = spsum.tile([128, 2, CHUNK], F32, tag="s", name="bvb_ps")
            nc.tensor.matmul(
                bvb_ps[:, 0, 0 : 2 * 128], ones_row[:],
                bv_row[:].rearrange("o h p -> o (h p)"),
                start=True, stop=True,
            )
            nc.scalar.copy(bv_bcast[:], bvb_ps[:, 0, 0 : 2 * 128])

        # --- attention chunk state ---
        class ChunkState:
            def __init__(self, c0):
                self.c0 = c0
                self.cs = slice(c0 * CHUNK, (c0 + 1) * CHUNK)
                self.acc0 = apsum.tile([128, CHUNK], F32, tag="acc", name="acc0")
                self.acc1 = apsum.tile([128, CHUNK], F32, tag="acc", name="acc1")
                # elementwise P-sum (fp8-in adds) rides the idle DVE
                self.psum = dpool.tile([128, 2, CHUNK], F16, tag="psum", name="psum")
                self.p_pairs = {}

        def s_pair(st, u):
            # two S tiles into the two banks of one wide PSUM slot; ONE
            # 1024-wide exp activation -> fp8 P pair
            p_pair = ppool.tile([128, 2, CHUNK], F8, tag="p", name="p")
            st.p_pairs[u] = p_pair
            sp = spsum.tile([128, 2, CHUNK], F32, tag="s", name="sp")
            for s2 in range(2):
                t = 2 * u + s2
                ts = slice(t * 128, (t + 1) * 128)
                nc.tensor.matmul(sp[:, s2, :], x2_sb[:, 0, ts], q_sb[:, 0, st.cs], start=True, stop=False)
                nc.tensor.matmul(sp[:, s2, :], x2_sb[:, 1, ts], q_sb[:, 1, st.cs], start=False, stop=True)
            nc.scalar.activation(p_pair[:], sp[:], mybir.ActivationFunctionType.Exp, scale=SCALE)

        def emit_pv(st, u):
            # fp8 DoubleRow: one matmul covers both key tiles of the pair
            first = u == 0
            p = st.p_pairs.pop(u)
            # stop stays False: the bias-fold matmuls close the acc groups
            nc.tensor.matmul(st.acc0[:], v_sb[:, u, :, 0:128], p[:],
                             start=first, stop=False, perf_mode=DR)
            nc.tensor.matmul(st.acc1[:], v_sb[:, u, :, 128:256], p[:],
                             start=first, stop=False, perf_mode=DR)
            if u == NPAIRS - 1:
                st.p_last = p
                return
            if first:
                nc.vector.tensor_copy(st.psum[:], p[:])
            else:
                nc.vector.tensor_add(st.psum[:], st.psum[:], p[:])
            if u == NPAIRS - 2:
                # fold the pair dim early, off the critical path
                st.acc_ra = dpool.tile([128, CHUNK], F32, tag="acc_ra", name="acc_ra")
                nc.vector.tensor_add(st.acc_ra[:], st.psum[:, 0, :], st.psum[:, 1, :])

        def flush_chunk(st):
            # trailing PV pairs (the final one waits on its exp), then the
            # two short DVE links that complete the P-sum
            for u in range(NPAIRS - PVTRAIL, NPAIRS):
                emit_pv(st, u)
            pf = dpool.tile([128, CHUNK], F16, tag="pf", name="pf")
            nc.vector.tensor_add(pf[:], st.p_last[:, 0, :], st.p_last[:, 1, :])
            st.acc_r = dpool.tile([128, CHUNK], MM, tag="acc_r", name="acc_r")
            nc.vector.tensor_add(st.acc_r[:], st.acc_ra[:], pf[:])

        # --- softmax tails. tail_a: denominator matmul + bias-fold matmuls
        # (acc_ct += bv_ct (x) den) closing the PV accumulation; reciprocal
        # straight from PSUM. tail_b: Pool-engine partition broadcast of
        # 1/den + normalize + out DMA. Chunks 0..2 run woven into the next
        # chunk's S stream; non-final out DMAs ride the Pool ring so the
        # final chunk's two half DMAs find empty Sync/Act rings. ---
        def tail_a(st):
            db = dpsum.tile([128, CHUNK], F32, tag="db", name="db_den")
            st.den = db[0:1, :]
            nc.tensor.matmul(st.den, ones_col[:], st.acc_r[:], start=True, stop=True)
            nc.tensor.matmul(st.acc0[:], bv_bcast[:, 0:128], st.acc_r[:], start=False, stop=True)
            nc.tensor.matmul(st.acc1[:], bv_bcast[:, 128:256], st.acc_r[:], start=False, stop=True)
            recip_f32 = dpool.tile([1, CHUNK], F32, tag="recip_f32", name="recip_f32")
            nc.vector.reciprocal_approx_fast(out=recip_f32[:], in_=st.den)
            st.recip_f32 = recip_f32

        def tail_b(st, final=False):
            bcast_sb = opool.tile([128, CHUNK], F32, tag="bcast_sb", name="bcast_sb")
            nc.gpsimd.partition_broadcast(bcast_sb[:], st.recip_f32[:])
            if final:
                # bias already folded; separate tiles per half, DMAs split
                # across the (empty) Sync and Activation rings
                o0 = opool.tile([128, CHUNK], F32, tag="o_f0", name="o_f0")
                nc.vector.tensor_mul(o0[:], st.acc0[:], bcast_sb[:])
                nc.sync.dma_start(
                    out=split_h(out[:, st.cs])[:, 0:1, :],
                    in_=o0[:].rearrange("p (o w) -> p o w", o=1),
                )
                o1 = opool.tile([128, CHUNK], F32, tag="o_f1", name="o_f1")
                nc.vector.tensor_mul(o1[:], st.acc1[:], bcast_sb[:])
                nc.scalar.dma_start(
                    out=split_h(out[:, st.cs])[:, 1:2, :],
                    in_=o1[:].rearrange("p (o w) -> p o w", o=1),
                )
            else:
                o2 = opool.tile([128, 2, CHUNK], F32, tag="o2", name="o2")
                for ct, acc in ((0, st.acc0), (1, st.acc1)):
                    nc.vector.tensor_mul(o2[:, ct, :], acc[:], bcast_sb[:])
                nc.gpsimd.dma_start(out=split_h(out[:, st.cs]), in_=o2[:])

        # ================= program =================
        # Prologue: V pairs + q projections, streamed behind the input DMA.
        vproj_pair(0)
        vproj_pair(1)
        qproj(0)
        for u in range(2, NPAIRS):
            vproj_pair(u)
        for c0 in range(1, NQ_CHUNKS):
            qproj(c0)
        bv_setup()

        # Main loop: chunks 0..3 in S pairs; the previous chunk's trailing
        # PV pairs and its tail are woven into pairs 1..3.
        prev = None
        for c0 in range(NQ_CHUNKS):
            st = ChunkState(c0)
            for u in range(NPAIRS):
                s_pair(st, u)
                if u == 1 and prev is not None:
                    flush_chunk(prev)
                if u == 2 and prev is not None:
                    tail_a(prev)
                if u == 3 and prev is not None:
                    tail_b(prev)
                    prev = None
                if u >= PVTRAIL:
                    emit_pv(st, u - PVTRAIL)
            prev = st

        # final chunk's tail is exposed: shortest possible chain
        flush_chunk(prev)
        tail_a(prev)
        tail_b(prev, final=True)

    nc.compile()
    return nc


def _presplit(m):
    """[256, w] -> [128, 2, w] partition-major (the SBUF layout), so DMA
    descriptors are large contiguous per-partition runs."""
    w = m.shape[1]
    return np.ascontiguousarray(m.reshape(2, 128, w).transpose(1, 0, 2))


def core_inputs(inputs, core):
    """Slice full-problem inputs for one core (numpy). Host-side weight
    fusion: A = Wq^T Wk and cq = Wk^T bq fold the K projection away."""
    b, h = core // 2, core % 2
    x1r = np.asarray(inputs["x1"], dtype=np.float32).reshape(B, C, N)
    x2r = np.asarray(inputs["x2"], dtype=np.float32).reshape(B, C, N)
    Wq = np.asarray(inputs["Wq"], dtype=np.float32)
    Wk = np.asarray(inputs["Wk"], dtype=np.float32)
    A = (Wq.T @ Wk).astype(np.float32)  # [ci, r]
    cqv = (Wk.T @ np.asarray(inputs["bq"], dtype=np.float32)).astype(np.float32)
    im = {
        "x1c": _presplit(x1r[b][:, h * NQ : (h + 1) * NQ]),
        "aT": _presplit(A),
        "wvT": _presplit(np.asarray(inputs["Wv"], dtype=np.float32).T),
        "cq": _presplit(cqv.reshape(C, 1)),
        "bv": np.asarray(inputs["bv"], dtype=np.float32).reshape(C, 1).copy(),
    }
    for g in range(4):
        im[f"x2{g}"] = _presplit(x2r[b][:, g * 1024 : (g + 1) * 1024])
    return im


_NC_CACHE = {}


def get_nc():
    if "nc" not in _NC_CACHE:
        _NC_CACHE["nc"] = build_nc()
    return _NC_CACHE["nc"]


def kernel(**inputs) -> np.ndarray:
    """Full-problem entry point: full inputs in, full [4,256,64,64] f32 out."""
    nc = get_nc()
    in_maps = [core_inputs(inputs, core) for core in range(8)]
    res = run_bass_kernel_spmd(nc, in_maps, list(range(8)))
    full = np.zeros((B, C, N), np.float32)
    for core in range(8):
        b, h = core // 2, core % 2
        full[b][:, h * NQ : (h + 1) * NQ] = res.results[core]["out"]
    return full.reshape(B, C, H, W)


# revision 22
# speedup vs baseline: 1.0108x; 1.0108x over previous
"""ConvCrossAttention Trainium2 kernel — self-contained.

Problem (B=4, C_in=C_out=256, H=W=64, N=4096):
  q = conv1x1(x1, Wq, bq); k = conv1x1(x2, Wk, bk); v = conv1x1(x2, Wv, bv)
  out = softmax(q^T k / sqrt(C)) @ v^T, back in conv layout [B, C, H, W].

Sharding: data-parallel over (batch, query-half) -> 8 NeuronCores.
Core c handles batch c//2, query rows (c%2)*2048 : (c%2+1)*2048, with the
full 4096-key context for that batch. No collectives.

Weight fusion (host side): softmax over keys is invariant to per-query
additive constants, so
  S_nm =(softmax) (A^T x1_n + c) . x2_m   with A = Wq^T Wk, c = Wk^T bq.
The K projection disappears entirely (raw x2 is the key matrix) and the
q projection uses the fused A instead of Wq. All inputs are pre-split on
the host to the [128 partitions, 2 halves, w] SBUF layout so every DMA
moves large contiguous per-partition runs (2KB descriptors starved
behind the 4-8KB x2 quarters otherwise).

Per-core program (everything SBUF-resident):
  Warm-up: fp32 dummy matmuls during the input-DMA head keep the PE busy
  so the HAM clock gate reaches 8/8 (2.4 GHz) before real work.
  DMA: three descriptor rings (Sync / Activation / Pool), each leading
  with its critical tensor: [aT, x1, x2 q4], [wv, x2 q2, bv], [cq, x2 q1,
  x2 q3].
  Prologue: all 16 V-pair projections (fp8 out) + the 4 q projections
  stream behind the DMA.
  Main loop: 4 query chunks, flash-style, in 2-key-tile pairs: S^T =
  x2^T q (PE, f32r, two banks of one wide PSUM slot), P = exp(S/16)
  (ACT, ONE 1024-wide activation per pair, fp8e4 out; |scores|/16 < ~5
  so no max-subtraction, p_max << 448), PV via fp8 DoubleRow matmuls
  (one matmul per 256 keys). P-sums ride the idle DVE (fp8-in adds;
  GpSimd is ~5x slower on fp8 and PE DoubleRow reductions cost a full
  matmul slot - both measured). The previous chunk's trailing PV pairs
  and its tail (denominator matmul, bv-fold matmuls closing the
  accumulation, reciprocal, Pool-engine partition broadcast, normalize,
  out-DMA) are woven into pairs 1..3 of the next chunk's S stream so the
  in-order PE queue never stalls.

S matmuls stay float32r (1 cyc/row); dropping scores to fp8 would cost
~1.7e-2 relative error (measured off-line) against the 2e-2 budget, while
fp8 P/V costs only ~1e-2. Softmax denominators use reciprocal_approx_fast
(~18-bit); inputs are sums of positive exps so its undefined edge cases
(0/denorm/inf) cannot occur.
"""

import sys

if "/opt/trn_rl_repo" not in sys.path:
    sys.path.insert(0, "/opt/trn_rl_repo")

from contextlib import ExitStack

import numpy as np

import concourse.bass as bass  # noqa: F401
import concourse.mybir as mybir
import concourse.tile as tile
from concourse import bacc
from concourse.bass_utils import run_bass_kernel_spmd

F32 = mybir.dt.float32
F32R = mybir.dt.float32r
F16 = mybir.dt.float16
F8 = mybir.dt.float8e4
DR = mybir.MatmulPerfMode.DoubleRow

B, C, H, W = 4, 256, 64, 64
N = H * W  # 4096
NQ = 2048  # queries per core (half a batch)
NK = 4096  # full key context
CHUNK = 512
NQ_CHUNKS = NQ // CHUNK
NK_TILES = NK // 128  # 32
NPAIRS = NK_TILES // 2  # 16 fp8 DoubleRow PV pairs
XQ = 1024  # x2 DMA quarter width
SCALE = 1.0 / 16.0  # C ** -0.5
WARMUP_MMS = 3  # fp32 dummy matmuls (~1.7us each) bridging the DMA head
PVTRAIL = 5  # PV pairs trail S pairs by this much
ADDTRAIL = 2  # DVE P-sum adds trail S pairs by this much


def build_nc():
    MM = F32R
    nc = bacc.Bacc(None, debug=False)

    # all pre-split host-side: [128, 2, w]; x1 in halves so two DMA
    # rings carry it concurrently (one 2MB tensor on one ring gates the
    # q projection at ~1/3 of the aggregate HBM bandwidth)
    x1a = nc.dram_tensor("x1a", [128, 2, NQ // 2], MM, kind="ExternalInput")
    x1b = nc.dram_tensor("x1b", [128, 2, NQ // 2], MM, kind="ExternalInput")
    x2g = [
        nc.dram_tensor(f"x2{g}", [128, 2, XQ], MM, kind="ExternalInput")
        for g in range(NK // XQ)
    ]
    at = nc.dram_tensor("aT", [128, 2, C], MM, kind="ExternalInput")  # A = Wq^T Wk
    wv = nc.dram_tensor("wvT", [128, 2, C], MM, kind="ExternalInput")
    cq = nc.dram_tensor("cq", [128, 2, 1], F32, kind="ExternalInput")  # Wk^T bq
    bv = nc.dram_tensor("bv", [C, 1], F32, kind="ExternalInput")
    out = nc.dram_tensor("out", [C, NQ], F32, kind="ExternalOutput")

    def split_h(ap):  # DRAM [256, w] -> [128, 2, w] (partition-first)
        return ap.rearrange("(h p) w -> p h w", p=128)

    with tile.TileContext(nc) as tc, ExitStack() as ctx:
        big = ctx.enter_context(tc.tile_pool(name="big", bufs=1))
        small = ctx.enter_context(tc.tile_pool(name="small", bufs=1))
        ppool = ctx.enter_context(tc.tile_pool(name="p", bufs=6))
        opool = ctx.enter_context(tc.tile_pool(name="o", bufs=2))
        dpool = ctx.enter_context(tc.tile_pool(name="d", bufs=2))
        # PSUM: 2 wide S slots (2 banks each) + 3 acc + 1 den = 8 banks
        spsum = ctx.enter_context(tc.tile_pool(name="spsum", bufs=2, space="PSUM"))
        apsum = ctx.enter_context(tc.tile_pool(name="apsum", bufs=3, space="PSUM"))
        dpsum = ctx.enter_context(tc.tile_pool(name="dpsum", bufs=1, space="PSUM"))

        # --- SBUF residents ---
        a_sb = small.tile([128, 2, C], MM, tag="a")
        wv_sb = small.tile([128, 2, C], MM, tag="wv")
        cq_sb = small.tile([128, 2, 1], F32, tag="cq")
        x1_sb = big.tile([128, 2, NQ], MM, tag="x1")
        x2_sb = big.tile([128, 2, NK], MM, tag="x2")
        q_sb = big.tile([128, 2, NQ], MM, tag="q")
        v_sb = big.tile([128, NPAIRS, 2, C], F8, tag="v")
        wu = small.tile([128, 512], F32, tag="wu")
        ones_col_f32 = small.tile([128, 1], F32, tag="ones_col_f32")
        ones_row_f32 = small.tile([1, 128], F32, tag="ones_row_f32")
        ones_col = small.tile([128, 1], MM, tag="ones_col")
        ones_row = small.tile([1, 128], MM, tag="ones_row")
        bv_row = small.tile([1, 2, 128], MM, tag="bv_row")
        bv_bcast = small.tile([128, 2 * 128], MM, tag="bv_bcast")

        # --- DMA triggers. Three rings (Sync / Activation HWDGE, Pool
        # SWDGE) drain in FIFO order each, so every ring leads with its
        # critical tensor. ---
        nc.gpsimd.memset(wu[:], 0.0)
        nc.gpsimd.dma_start(out=cq_sb[:], in_=cq[:])
        nc.gpsimd.dma_start(out=x2_sb[:, :, 0:XQ], in_=x2g[0][:])
        nc.gpsimd.memset(ones_col_f32[:], 1.0)
        nc.gpsimd.memset(ones_row_f32[:], 1.0)
        nc.vector.tensor_copy(ones_col[:], ones_col_f32[:])
        nc.vector.tensor_copy(ones_row[:], ones_row_f32[:])
        nc.gpsimd.dma_start(out=x2_sb[:, :, XQ : 2 * XQ], in_=x2g[1][:])

        nc.sync.dma_start(out=a_sb[:], in_=at[:])
        nc.sync.dma_start(out=x1_sb[:, :, 0 : NQ // 2], in_=x1a[:])
        nc.sync.dma_start(out=x2_sb[:, :, 2 * XQ : 3 * XQ], in_=x2g[2][:])

        nc.scalar.dma_start(out=wv_sb[:], in_=wv[:])
        nc.scalar.dma_start(out=x1_sb[:, :, NQ // 2 : NQ], in_=x1b[:])
        nc.scalar.dma_start(out=x2_sb[:, :, 3 * XQ : 4 * XQ], in_=x2g[3][:])
        nc.scalar.dma_start(
            out=bv_row[:], in_=bv[:, :].rearrange("(h p) o -> o h p", p=128).bitcast(F32R)
        )

        # --- HAM warm-up: fp32 dummy matmuls (4 cyc/row, ~1.7us each)
        # with no input dependency bridge the DMA head so the PE reaches
        # the 8/8 clock before, and stays busy until, real work starts ---
        for _ in range(WARMUP_MMS):
            wup = dpsum.tile([128, 512], F32, tag="db", name="wup")
            nc.tensor.matmul(wup[:], wu[:, 0:128], wu[:], start=True, stop=True)

        # --- projection helpers (prologue; share the wide S slots one
        # bank at a time) ---
        def qproj(c0):
            cs = slice(c0 * CHUNK, (c0 + 1) * CHUNK)
            for ct in range(2):
                qp = spsum.tile([128, 2, CHUNK], F32, tag="s", name="qp")
                cts = slice(ct * 128, (ct + 1) * 128)
                nc.tensor.matmul(qp[:, 0, :], a_sb[:, 0, cts], x1_sb[:, 0, cs], start=True, stop=False)
                nc.tensor.matmul(qp[:, 0, :], a_sb[:, 1, cts], x1_sb[:, 1, cs], start=False, stop=True)
                nc.vector.tensor_scalar_add(q_sb[:, ct, cs], qp[:, 0, :], cq_sb[:, ct, :])

        def vproj_pair(u):
            # two 128-key tiles share one PSUM bank -> one wide fp8 copy out
            vp = spsum.tile([128, 2, C], F32, tag="s", name="vp")
            for s2 in range(2):
                t = 2 * u + s2
                ts_ = slice(t * 128, (t + 1) * 128)
                nc.tensor.matmul(
                    vp[:, s2, :], x2_sb[:, 0, ts_], wv_sb[:, 0, :],
                    start=(s2 == 0), stop=False,
                )
                nc.tensor.matmul(
                    vp[:, s2, :], x2_sb[:, 1, ts_], wv_sb[:, 1, :],
                    start=False, stop=(s2 == 1),
                )
            if u % 2 == 0:
                nc.scalar.copy(v_sb[:, u, :, :], vp[:])
            else:
                nc.vector.tensor_copy(v_sb[:, u, :, :], vp[:])

        def bv_setup():
            # bv broadcast to all partitions: bias-fold matmul stationary
            # (acc_ct += bv_ct (x) den, so no per-half DVE bias add needed)
            bvb_ps = spsum.tile([128, 2, CHUNK], F32, tag="s", name="bvb_ps")
            nc.tensor.matmul(
                bvb_ps[:, 0, 0 : 2 * 128], ones_row[:],
                bv_row[:].rearrange("o h p -> o (h p)"),
                start=True, stop=True,
            )
            nc.scalar.copy(bv_bcast[:], bvb_ps[:, 0, 0 : 2 * 128])

        # --- attention chunk state ---
        class ChunkState:
            def __init__(self, c0):
                self.c0 = c0
                self.cs = slice(c0 * CHUNK, (c0 + 1) * CHUNK)
                self.acc0 = apsum.tile([128, CHUNK], F32, tag="acc", name="acc0")
                self.acc1 = apsum.tile([128, CHUNK], F32, tag="acc", name="acc1")
                # elementwise P-sum (fp8-in adds) rides the idle DVE
                self.psum = dpool.tile([128, 2, CHUNK], F16, tag="psum", name="psum")
                self.p_pairs = {}

        def s_pair(st, u):
            # two S tiles into the two banks of one wide PSUM slot; ONE
            # 1024-wide exp activation -> fp8 P pair
            p_pair = ppool.tile([128, 2, CHUNK], F8, tag="p", name="p")
            st.p_pairs[u] = p_pair
            sp = spsum.tile([128, 2, CHUNK], F32, tag="s", name="sp")
            for s2 in range(2):
                t = 2 * u + s2
                ts = slice(t * 128, (t + 1) * 128)
                nc.tensor.matmul(sp[:, s2, :], x2_sb[:, 0, ts], q_sb[:, 0, st.cs], start=True, stop=False)
                nc.tensor.matmul(sp[:, s2, :], x2_sb[:, 1, ts], q_sb[:, 1, st.cs], start=False, stop=True)
            nc.scalar.activation(p_pair[:], sp[:], mybir.ActivationFunctionType.Exp, scale=SCALE)

        def emit_pv(st, u):
            # fp8 DoubleRow: one matmul covers both key tiles of the pair
            first = u == 0
            p = st.p_pairs[u]
            # stop stays False: the bias-fold matmuls close the acc groups
            nc.tensor.matmul(st.acc0[:], v_sb[:, u, :, 0:128], p[:],
                             start=first, stop=False, perf_mode=DR)
            nc.tensor.matmul(st.acc1[:], v_sb[:, u, :, 128:256], p[:],
                             start=first, stop=False, perf_mode=DR)

        def psum_add(st, u):
            # DVE P-sum, decoupled from the PV stream so the chunk
            # boundary never waits on a backlog of adds. Pairs 0..13 go
            # into the elementwise accumulator; 14 and 15 get their own
            # pair-folds so only ~0.7us of DVE chain follows the last exp.
            p = st.p_pairs[u]
            if u == 0:
                nc.vector.tensor_copy(st.psum[:], p[:])
            elif u <= NPAIRS - 3:
                nc.vector.tensor_add(st.psum[:], st.psum[:], p[:])
                if u == NPAIRS - 3:
                    st.acc_ra = dpool.tile([128, CHUNK], F32, tag="acc_ra", name="acc_ra")
                    nc.vector.tensor_add(st.acc_ra[:], st.psum[:, 0, :], st.psum[:, 1, :])
            else:
                pf = dpool.tile([128, CHUNK], F16, tag="pf", name="pf", bufs=2)
                nc.vector.tensor_add(pf[:], p[:, 0, :], p[:, 1, :])
                if u == NPAIRS - 2:
                    st.pf14 = pf
                else:
                    st.pf15 = pf

        def flush_chunk(st):
            # trailing PV pairs (the final one waits on its exp), then the
            # short DVE links that complete the P-sum
            for u in range(NPAIRS - PVTRAIL, NPAIRS):
                emit_pv(st, u)
            psum_add(st, NPAIRS - 2)
            psum_add(st, NPAIRS - 1)
            acc_rb = dpool.tile([128, CHUNK], F32, tag="acc_rb", name="acc_rb")
            nc.vector.tensor_add(acc_rb[:], st.pf14[:], st.pf15[:])
            st.acc_r = dpool.tile([128, CHUNK], MM, tag="acc_r", name="acc_r")
            nc.vector.tensor_add(st.acc_r[:], st.acc_ra[:], acc_rb[:])

        # --- softmax tails. tail_a: denominator matmul + bias-fold matmuls
        # (acc_ct += bv_ct (x) den) closing the PV accumulation; reciprocal
        # straight from PSUM. tail_b: Pool-engine partition broadcast of
        # 1/den + normalize + out DMA. Chunks 0..2 run woven into the next
        # chunk's S stream; non-final out DMAs ride the Pool ring so the
        # final chunk's two half DMAs find empty Sync/Act rings. ---
        def tail_a(st):
            db = dpsum.tile([128, CHUNK], F32, tag="db", name="db_den")
            st.den = db[0:1, :]
            nc.tensor.matmul(st.den, ones_col[:], st.acc_r[:], start=True, stop=True)
            nc.tensor.matmul(st.acc0[:], bv_bcast[:, 0:128], st.acc_r[:], start=False, stop=True)
            nc.tensor.matmul(st.acc1[:], bv_bcast[:, 128:256], st.acc_r[:], start=False, stop=True)
            recip_f32 = dpool.tile([1, CHUNK], F32, tag="recip_f32", name="recip_f32")
            nc.vector.reciprocal_approx_fast(out=recip_f32[:], in_=st.den)
            st.recip_f32 = recip_f32

        def tail_b(st, final=False):
            bcast_sb = opool.tile([128, CHUNK], F32, tag="bcast_sb", name="bcast_sb")
            nc.gpsimd.partition_broadcast(bcast_sb[:], st.recip_f32[:])
            if final:
                # bias already folded; separate tiles per half, DMAs split
                # across the (empty) Sync and Activation rings
                o0 = opool.tile([128, CHUNK], F32, tag="o_f0", name="o_f0")
                nc.vector.tensor_mul(o0[:], st.acc0[:], bcast_sb[:])
                nc.sync.dma_start(
                    out=split_h(out[:, st.cs])[:, 0:1, :],
                    in_=o0[:].rearrange("p (o w) -> p o w", o=1),
                )
                o1 = opool.tile([128, CHUNK], F32, tag="o_f1", name="o_f1")
                nc.vector.tensor_mul(o1[:], st.acc1[:], bcast_sb[:])
                nc.scalar.dma_start(
                    out=split_h(out[:, st.cs])[:, 1:2, :],
                    in_=o1[:].rearrange("p (o w) -> p o w", o=1),
                )
            else:
                o2 = opool.tile([128, 2, CHUNK], F32, tag="o2", name="o2")
                for ct, acc in ((0, st.acc0), (1, st.acc1)):
                    nc.vector.tensor_mul(o2[:, ct, :], acc[:], bcast_sb[:])
                nc.gpsimd.dma_start(out=split_h(out[:, st.cs]), in_=o2[:])

        # ================= program =================
        # Prologue: V pairs + q projections, streamed behind the input DMA.
        vproj_pair(0)
        vproj_pair(1)
        qproj(0)
        for u in range(2, NPAIRS):
            vproj_pair(u)
        for c0 in range(1, NQ_CHUNKS):
            qproj(c0)
        bv_setup()

        # Main loop: chunks 0..3 in S pairs; the previous chunk's trailing
        # PV pairs and its tail are woven into pairs 1..3.
        prev = None
        for c0 in range(NQ_CHUNKS):
            st = ChunkState(c0)
            for u in range(NPAIRS):
                s_pair(st, u)
                if u == 1 and prev is not None:
                    flush_chunk(prev)
                if u == 3 and prev is not None:
                    tail_a(prev)
                if u == 4 and prev is not None:
                    tail_b(prev)
                    prev = None
                if u >= ADDTRAIL:
                    psum_add(st, u - ADDTRAIL)
                if u >= PVTRAIL:
                    emit_pv(st, u - PVTRAIL)
            prev = st

        # final chunk's tail is exposed: shortest possible chain
        flush_chunk(prev)
        tail_a(prev)
        tail_b(prev, final=True)

    nc.compile()
    return nc


def _presplit(m):
    """[256, w] -> [128, 2, w] partition-major (the SBUF layout), so DMA
    descriptors are large contiguous per-partition runs."""
    w = m.shape[1]
    return np.ascontiguousarray(m.reshape(2, 128, w).transpose(1, 0, 2))


def core_inputs(inputs, core):
    """Slice full-problem inputs for one core (numpy). Host-side weight
    fusion: A = Wq^T Wk and cq = Wk^T bq fold the K projection away."""
    b, h = core // 2, core % 2
    x1r = np.asarray(inputs["x1"], dtype=np.float32).reshape(B, C, N)
    x2r = np.asarray(inputs["x2"], dtype=np.float32).reshape(B, C, N)
    Wq = np.asarray(inputs["Wq"], dtype=np.float32)
    Wk = np.asarray(inputs["Wk"], dtype=np.float32)
    A = (Wq.T @ Wk).astype(np.float32)  # [ci, r]
    cqv = (Wk.T @ np.asarray(inputs["bq"], dtype=np.float32)).astype(np.float32)
    x1c = x1r[b][:, h * NQ : (h + 1) * NQ]
    im = {
        "x1a": _presplit(x1c[:, 0 : NQ // 2]),
        "x1b": _presplit(x1c[:, NQ // 2 : NQ]),
        "aT": _presplit(A),
        "wvT": _presplit(np.asarray(inputs["Wv"], dtype=np.float32).T),
        "cq": _presplit(cqv.reshape(C, 1)),
        "bv": np.asarray(inputs["bv"], dtype=np.float32).reshape(C, 1).copy(),
    }
    for g in range(4):
        im[f"x2{g}"] = _presplit(x2r[b][:, g * 1024 : (g + 1) * 1024])
    return im


_NC_CACHE = {}


def get_nc():
    if "nc" not in _NC_CACHE:
        _NC_CACHE["nc"] = build_nc()
    return _NC_CACHE["nc"]


def kernel(**inputs) -> np.ndarray:
    """Full-problem entry point: full inputs in, full [4,256,64,64] f32 out."""
    nc = get_nc()
    in_maps = [core_inputs(inputs, core) for core in range(8)]
    res = run_bass_kernel_spmd(nc, in_maps, list(range(8)))
    full = np.zeros((B, C, N), np.float32)
    for core in range(8):
        b, h = core // 2, core % 2
        full[b][:, h * NQ : (h + 1) * NQ] = res.results[core]["out"]
    return full.reshape(B, C, H, W)


# revision 25
# speedup vs baseline: 1.1327x; 1.1206x over previous
"""ConvCrossAttention Trainium2 kernel — self-contained.

Problem (B=4, C_in=C_out=256, H=W=64, N=4096):
  q = conv1x1(x1, Wq, bq); k = conv1x1(x2, Wk, bk); v = conv1x1(x2, Wv, bv)
  out = softmax(q^T k / sqrt(C)) @ v^T, back in conv layout [B, C, H, W].

Sharding: data-parallel over (batch, query-half) -> 8 NeuronCores.
Core c handles batch c//2, query rows (c%2)*2048 : (c%2+1)*2048, with the
full 4096-key context for that batch. No collectives.

Weight fusion (host side): softmax over keys is invariant to per-query
additive constants, so
  S_nm =(softmax) (A^T x1_n + c) . x2_m   with A = Wq^T Wk, c = Wk^T bq.
The K projection disappears entirely (raw x2 is the key matrix) and the
q projection uses the fused A instead of Wq.

Per-core program (everything SBUF-resident):
  Warm-up: fp32 dummy matmuls during the input-DMA head keep the PE busy
  so the HAM clock gate reaches 8/8 (2.4 GHz) before real work.
  DMA: three independent descriptor rings (Sync / Activation / Pool) carry
  [aT, x1], [wv, x2 cols 0:2048, bv] and [cq, x2 cols 2048:4096]; inputs
  land in consumption order at the ~358 GB/s aggregate limit.
  Phase A (streamed behind the DMA, woven into chunk-0 attention): per
  512-col x2 chunk j: V^T projection into fp8 pairs, then chunk-0 S tiles
  with PV trailing; q projections for chunks 1..3 woven in later.
  Phase B: chunks 1..3, flash-style: S^T = x2^T q (PE, f32r), P = exp(S/16)
  (ACT, fp8e4 out; |scores|/16 < ~5 so no max-subtraction, p_max << 448),
  PV accumulated in PSUM via fp8 DoubleRow matmuls (2 key-tiles per
  instruction, 0.5 cyc/row), P-sums split Pool/DVE. The previous chunk's
  last PV pair (which waits on its exp) and its softmax tail are woven
  INTO the next chunk's S stream so the in-order PE queue never stalls.

S matmuls stay float32r (1 cyc/row); dropping scores to fp8 would cost
~1.7e-2 relative error (measured off-line) against the 2e-2 budget, while
fp8 P/V costs only ~1e-2. Softmax denominators use reciprocal_approx_fast
(~18-bit); inputs are sums of positive exps so its undefined edge cases
(0/denorm/inf) cannot occur.
"""

import sys

if "/opt/trn_rl_repo" not in sys.path:
    sys.path.insert(0, "/opt/trn_rl_repo")

from contextlib import ExitStack

import numpy as np

import concourse.bass as bass  # noqa: F401
import concourse.mybir as mybir
import concourse.tile as tile
from concourse import bacc
from concourse.bass_utils import run_bass_kernel_spmd

F32 = mybir.dt.float32
F32R = mybir.dt.float32r
F16 = mybir.dt.float16
F8 = mybir.dt.float8e4
DR = mybir.MatmulPerfMode.DoubleRow

B, C, H, W = 4, 256, 64, 64
N = H * W  # 4096
NQ = 2048  # queries per core (half a batch)
NK = 4096  # full key context
CHUNK = 512
NQ_CHUNKS = NQ // CHUNK
NK_TILES = NK // 128  # 32
NPAIRS = NK_TILES // 2  # 16 fp8 DoubleRow PV pairs
SCALE = 1.0 / 16.0  # C ** -0.5
WARMUP_MMS = 3  # fp32 dummy matmuls (~1.7us each) bridging the DMA head


def build_nc():
    MM = F32R
    nc = bacc.Bacc(None, debug=False)

    # all pre-split host-side to [128, 2, w] so DMA descriptors are
    # large contiguous per-partition runs; x1 chunk 0 separate so the
    # first q projection is gated by 0.75MB, not 2MB, of sync-ring data
    x1a = nc.dram_tensor("x1a", [128, 2, CHUNK], MM, kind="ExternalInput")
    x1b = nc.dram_tensor("x1b", [128, 2, NQ - CHUNK], MM, kind="ExternalInput")
    x2g = [
        nc.dram_tensor(f"x2{g}", [128, 2, 1024], MM, kind="ExternalInput")
        for g in range(4)
    ]
    at = nc.dram_tensor("aT", [128, 2, C], MM, kind="ExternalInput")  # A = Wq^T Wk
    wv = nc.dram_tensor("wvT", [128, 2, C], MM, kind="ExternalInput")
    cq = nc.dram_tensor("cq", [128, 2, 1], F32, kind="ExternalInput")  # Wk^T bq
    bv = nc.dram_tensor("bv", [C, 1], F32, kind="ExternalInput")
    out = nc.dram_tensor("out", [C, NQ], F32, kind="ExternalOutput")

    def split_h(ap):  # DRAM [256, w] -> [128, 2, w] (partition-first)
        return ap.rearrange("(h p) w -> p h w", p=128)

    with tile.TileContext(nc) as tc, ExitStack() as ctx:
        big = ctx.enter_context(tc.tile_pool(name="big", bufs=1))
        small = ctx.enter_context(tc.tile_pool(name="small", bufs=1))
        ppool = ctx.enter_context(tc.tile_pool(name="p", bufs=4))
        opool = ctx.enter_context(tc.tile_pool(name="o", bufs=2))
        dpool = ctx.enter_context(tc.tile_pool(name="d", bufs=2))
        spsum = ctx.enter_context(tc.tile_pool(name="spsum", bufs=3, space="PSUM"))
        apsum = ctx.enter_context(tc.tile_pool(name="apsum", bufs=4, space="PSUM"))
        dpsum = ctx.enter_context(tc.tile_pool(name="dpsum", bufs=1, space="PSUM"))

        # --- SBUF residents ---
        a_sb = small.tile([128, 2, C], MM, tag="a")
        wv_sb = small.tile([128, 2, C], MM, tag="wv")
        cq_sb = small.tile([128, 2, 1], F32, tag="cq")
        x1_sb = big.tile([128, 2, NQ], MM, tag="x1")
        x2_sb = big.tile([128, 2, NK], MM, tag="x2")
        q_sb = big.tile([128, 2, NQ], MM, tag="q")
        v_sb = big.tile([128, NPAIRS, 2, C], F8, tag="v")
        wu = small.tile([128, 512], F32, tag="wu")
        ones_col_f32 = small.tile([128, 1], F32, tag="ones_col_f32")
        ones_row_f32 = small.tile([1, 128], F32, tag="ones_row_f32")
        ones_col = small.tile([128, 1], MM, tag="ones_col")
        ones_row = small.tile([1, 128], MM, tag="ones_row")
        bv_row = small.tile([1, 2, 128], MM, tag="bv_row")
        bv_bcast = small.tile([128, 2 * 128], MM, tag="bv_bcast")

        # memsets early on Pool; f32r/fp8 copies round on write (DVE)
        nc.gpsimd.memset(wu[:], 0.0)
        nc.gpsimd.memset(ones_col_f32[:], 1.0)
        nc.gpsimd.memset(ones_row_f32[:], 1.0)
        nc.vector.tensor_copy(ones_col[:], ones_col_f32[:])
        nc.vector.tensor_copy(ones_row[:], ones_row_f32[:])

        # --- DMA triggers. Three rings (Sync / Activation HWDGE, Pool
        # SWDGE) drain in FIFO order each, so every ring leads with its
        # critical tensor. x2 is split in 1024-col quarters for 4KB
        # descriptor runs; x1 chunk 0 is split out so the q projection
        # can start ~2us earlier. ---
        nc.sync.dma_start(out=a_sb[:], in_=at[:])
        nc.sync.dma_start(out=x1_sb[:, :, 0:CHUNK], in_=x1a[:])
        nc.sync.dma_start(out=x1_sb[:, :, CHUNK:NQ], in_=x1b[:])
        nc.scalar.dma_start(out=wv_sb[:], in_=wv[:])
        for g in range(2):
            nc.scalar.dma_start(out=x2_sb[:, :, g * 1024 : (g + 1) * 1024], in_=x2g[g][:])
        nc.scalar.dma_start(
            out=bv_row[:], in_=bv[:, :].rearrange("(h p) o -> o h p", p=128).bitcast(F32R)
        )
        nc.gpsimd.dma_start(out=cq_sb[:], in_=cq[:])
        for g in range(2, 4):
            nc.gpsimd.dma_start(out=x2_sb[:, :, g * 1024 : (g + 1) * 1024], in_=x2g[g][:])

        # --- HAM warm-up: fp32 dummy matmuls (4 cyc/row, ~1.7us each)
        # with no input dependency bridge the DMA head so the PE reaches
        # the 8/8 clock before, and stays busy until, real work starts ---
        for _ in range(WARMUP_MMS):
            wup = dpsum.tile([128, 512], F32, tag="db", name="wup")
            nc.tensor.matmul(wup[:], wu[:, 0:128], wu[:], start=True, stop=True)

        # --- projection helpers ---
        def qproj(c0):
            cs = slice(c0 * CHUNK, (c0 + 1) * CHUNK)
            for ct in range(2):
                qp = spsum.tile([128, CHUNK], F32, tag="s", name="qp")
                cts = slice(ct * 128, (ct + 1) * 128)
                nc.tensor.matmul(qp[:], a_sb[:, 0, cts], x1_sb[:, 0, cs], start=True, stop=False)
                nc.tensor.matmul(qp[:], a_sb[:, 1, cts], x1_sb[:, 1, cs], start=False, stop=True)
                nc.vector.tensor_scalar_add(q_sb[:, ct, cs], qp[:], cq_sb[:, ct, :])

        def vproj_pair(u):
            # two 128-key tiles share one PSUM bank -> one wide fp8 copy out
            vp = spsum.tile([128, 2, C], F32, tag="s", name="vp")
            for s2 in range(2):
                t = 2 * u + s2
                ts_ = slice(t * 128, (t + 1) * 128)
                nc.tensor.matmul(
                    vp[:, s2, :], x2_sb[:, 0, ts_], wv_sb[:, 0, :],
                    start=(s2 == 0), stop=False,
                )
                nc.tensor.matmul(
                    vp[:, s2, :], x2_sb[:, 1, ts_], wv_sb[:, 1, :],
                    start=False, stop=(s2 == 1),
                )
            if u % 2 == 0:
                nc.scalar.copy(v_sb[:, u, :, :], vp[:])
            else:
                nc.vector.tensor_copy(v_sb[:, u, :, :], vp[:])

        # --- attention chunk state ---
        class ChunkState:
            def __init__(self, c0):
                self.c0 = c0
                self.cs = slice(c0 * CHUNK, (c0 + 1) * CHUNK)
                self.acc0 = apsum.tile([128, CHUNK], F32, tag="acc", name="acc0")
                self.acc1 = apsum.tile([128, CHUNK], F32, tag="acc", name="acc1")
                # elementwise P-sum (fp8-in adds) rides the idle DVE;
                # pairs 14/15 get their own folds so only ~1us of DVE
                # chain follows the last exp
                self.psum = dpool.tile([128, 2, CHUNK], F16, tag="psum", name="psum")
                self.p_pairs = {}

        def s_tile(st, t):
            u, s2 = divmod(t, 2)
            if s2 == 0:
                st.p_pairs[u] = ppool.tile([128, 2, CHUNK], F8, tag="p", name="p")
            ts = slice(t * 128, (t + 1) * 128)
            sp = spsum.tile([128, CHUNK], F32, tag="s", name="sp")
            nc.tensor.matmul(sp[:], x2_sb[:, 0, ts], q_sb[:, 0, st.cs], start=True, stop=False)
            nc.tensor.matmul(sp[:], x2_sb[:, 1, ts], q_sb[:, 1, st.cs], start=False, stop=True)
            nc.scalar.activation(
                st.p_pairs[u][:, s2, :], sp[:], mybir.ActivationFunctionType.Exp, scale=SCALE
            )

        def emit_pv(st, u):
            # fp8 DoubleRow: one matmul covers both key tiles of the pair
            first = u == 0
            p = st.p_pairs[u]
            # stop stays False: the bias-fold matmuls close the acc groups
            nc.tensor.matmul(st.acc0[:], v_sb[:, u, :, 0:128], p[:],
                             start=first, stop=False, perf_mode=DR)
            nc.tensor.matmul(st.acc1[:], v_sb[:, u, :, 128:256], p[:],
                             start=first, stop=False, perf_mode=DR)

        def psum_add(st, u):
            # DVE P-sum, decoupled from the PV stream (fp8-in adds are
            # ~0.7us; GpSimd is ~5x slower on fp8, and a PE DoubleRow
            # reduction costs a full matmul slot - both measured)
            p = st.p_pairs[u]
            if u == 0:
                nc.vector.tensor_copy(st.psum[:], p[:])
            elif u <= NPAIRS - 3:
                nc.vector.tensor_add(st.psum[:], st.psum[:], p[:])
                if u == NPAIRS - 3:
                    st.acc_ra = dpool.tile([128, CHUNK], F32, tag="acc_ra", name="acc_ra")
                    nc.vector.tensor_add(st.acc_ra[:], st.psum[:, 0, :], st.psum[:, 1, :])
            else:
                pf = dpool.tile([128, CHUNK], F16, tag="pf", name="pf", bufs=2)
                nc.vector.tensor_add(pf[:], p[:, 0, :], p[:, 1, :])
                if u == NPAIRS - 2:
                    st.pf14 = pf
                else:
                    st.pf15 = pf

        def flush_chunk(st):
            # last PV pair (waits on exp of tile 31), then the short DVE
            # links completing the P-sum
            emit_pv(st, NPAIRS - 1)
            psum_add(st, NPAIRS - 2)
            psum_add(st, NPAIRS - 1)
            acc_rb = dpool.tile([128, CHUNK], F32, tag="acc_rb", name="acc_rb")
            nc.vector.tensor_add(acc_rb[:], st.pf14[:], st.pf15[:])
            st.acc_r = dpool.tile([128, CHUNK], MM, tag="acc_r", name="acc_r")
            nc.vector.tensor_add(st.acc_r[:], st.acc_ra[:], acc_rb[:])

        # --- softmax tails. tail_a: bias-fold matmuls (acc_ct += bv_ct (x)
        # den) close the PV accumulation groups. tail_b: broadcast +
        # normalize + out DMA. For chunks 0..2 these run woven into the
        # next chunk's S stream; non-final out DMAs ride the Pool ring so
        # the final chunk's two half DMAs find empty Sync/Act rings. ---
        def bv_setup():
            # bv broadcast to all partitions: bias-fold matmul stationary
            # (acc_ct += bv_ct (x) den, so no per-half DVE bias add needed)
            bvb_ps = spsum.tile([128, CHUNK], F32, tag="s", name="bvb_ps")
            nc.tensor.matmul(
                bvb_ps[:, 0 : 2 * 128], ones_row[:],
                bv_row[:].rearrange("o h p -> o (h p)"),
                start=True, stop=True,
            )
            nc.scalar.copy(bv_bcast[:], bvb_ps[:, 0 : 2 * 128])

        def tail_a(st):
            db = dpsum.tile([128, CHUNK], F32, tag="db", name="db_den")
            st.den = db[0:1, :]
            nc.tensor.matmul(st.den, ones_col[:], st.acc_r[:], start=True, stop=True)
            nc.tensor.matmul(st.acc0[:], bv_bcast[:, 0:128], st.acc_r[:], start=False, stop=True)
            nc.tensor.matmul(st.acc1[:], bv_bcast[:, 128:256], st.acc_r[:], start=False, stop=True)
            recip_f32 = dpool.tile([1, CHUNK], F32, tag="recip_f32", name="recip_f32")
            nc.vector.reciprocal_approx_fast(out=recip_f32[:], in_=st.den)
            st.recip_f32 = recip_f32

        def tail_b(st, final=False):
            bcast_sb = opool.tile([128, CHUNK], F32, tag="bcast_sb", name="bcast_sb")
            nc.gpsimd.partition_broadcast(bcast_sb[:], st.recip_f32[:])
            if final:
                # bias already folded; separate tiles per half, DMAs split
                # across the (empty) Sync and Activation rings
                o0 = opool.tile([128, CHUNK], F32, tag="o_f0", name="o_f0")
                nc.vector.tensor_mul(o0[:], st.acc0[:], bcast_sb[:])
                nc.sync.dma_start(
                    out=split_h(out[:, st.cs])[:, 0:1, :],
                    in_=o0[:].rearrange("p (o w) -> p o w", o=1),
                )
                o1 = opool.tile([128, CHUNK], F32, tag="o_f1", name="o_f1")
                nc.vector.tensor_mul(o1[:], st.acc1[:], bcast_sb[:])
                nc.scalar.dma_start(
                    out=split_h(out[:, st.cs])[:, 1:2, :],
                    in_=o1[:].rearrange("p (o w) -> p o w", o=1),
                )
            else:
                o2 = opool.tile([128, 2, CHUNK], F32, tag="o2", name="o2")
                for ct, acc in ((0, st.acc0), (1, st.acc1)):
                    nc.vector.tensor_mul(o2[:, ct, :], acc[:], bcast_sb[:])
                nc.gpsimd.dma_start(out=split_h(out[:, st.cs]), in_=o2[:])

        def maybe_pv(st, t):
            # PV pair u-1 goes out once pair u's exps are both issued;
            # P-sum adds trail one pair further
            if t % 2 == 1:
                u = (t - 1) // 2
                if u >= 1:
                    emit_pv(st, u - 1)  # pairs 0..14; pair 15 in flush
                if u >= 2:
                    psum_add(st, u - 2)  # pairs 0..13; 14/15 in flush

        # ================= program =================
        # Phase A: V projections + chunk-0 attention stream behind the x2
        # DMA; q projections woven in as x1 lands.
        st0 = ChunkState(0)
        qproj(0)
        for j in range(NK // 512):
            vproj_pair(2 * j)
            vproj_pair(2 * j + 1)
            if j == 3:
                bv_setup()
            if j >= 5:
                qproj(j - 4)  # chunks 1..3 at j=5,6,7
            for i in range(4):
                t = 4 * j + i
                s_tile(st0, t)
                maybe_pv(st0, t)
        prev = st0

        # Phase B: chunks 1..3; the previous chunk's last PV pairs and its
        # tail are woven into this chunk's S stream (flush after tile 1,
        # tail_a after tile 4, tail_b after tile 8).
        for c0 in range(1, NQ_CHUNKS):
            st = ChunkState(c0)
            for t in range(NK_TILES):
                s_tile(st, t)
                if t == 1 and prev is not None:
                    flush_chunk(prev)
                if t == 4 and prev is not None:
                    tail_a(prev)
                if t == 8 and prev is not None:
                    tail_b(prev)
                    prev = None
                maybe_pv(st, t)
            prev = st

        # final chunk's tail is exposed: shortest possible chain
        flush_chunk(prev)
        tail_a(prev)
        tail_b(prev, final=True)

    nc.compile()
    return nc


def _presplit(m):
    """[256, w] -> [128, 2, w] partition-major (the SBUF layout), so DMA
    descriptors are large contiguous per-partition runs."""
    w = m.shape[1]
    return np.ascontiguousarray(m.reshape(2, 128, w).transpose(1, 0, 2))


def core_inputs(inputs, core):
    """Slice full-problem inputs for one core (numpy). Host-side weight
    fusion: A = Wq^T Wk and cq = Wk^T bq fold the K projection away."""
    b, h = core // 2, core % 2
    x1r = np.asarray(inputs["x1"], dtype=np.float32).reshape(B, C, N)
    x2r = np.asarray(inputs["x2"], dtype=np.float32).reshape(B, C, N)
    Wq = np.asarray(inputs["Wq"], dtype=np.float32)
    Wk = np.asarray(inputs["Wk"], dtype=np.float32)
    A = (Wq.T @ Wk).astype(np.float32)  # [ci, r]
    cqv = (Wk.T @ np.asarray(inputs["bq"], dtype=np.float32)).astype(np.float32)
    x1c = x1r[b][:, h * NQ : (h + 1) * NQ]
    im = {
        "x1a": _presplit(x1c[:, 0:CHUNK]),
        "x1b": _presplit(x1c[:, CHUNK:NQ]),
        "aT": _presplit(A),
        "wvT": _presplit(np.asarray(inputs["Wv"], dtype=np.float32).T),
        "cq": _presplit(cqv.reshape(C, 1)),
        "bv": np.asarray(inputs["bv"], dtype=np.float32).reshape(C, 1).copy(),
    }
    for g in range(4):
        im[f"x2{g}"] = _presplit(x2r[b][:, g * 1024 : (g + 1) * 1024])
    return im


_NC_CACHE = {}


def get_nc():
    if "nc" not in _NC_CACHE:
        _NC_CACHE["nc"] = build_nc()
    return _NC_CACHE["nc"]


def kernel(**inputs) -> np.ndarray:
    """Full-problem entry point: full inputs in, full [4,256,64,64] f32 out."""
    nc = get_nc()
    in_maps = [core_inputs(inputs, core) for core in range(8)]
    res = run_bass_kernel_spmd(nc, in_maps, list(range(8)))
    full = np.zeros((B, C, N), np.float32)
    for core in range(8):
        b, h = core // 2, core % 2
        full[b][:, h * NQ : (h + 1) * NQ] = res.results[core]["out"]
    return full.reshape(B, C, H, W)
